# revision 5
# baseline (speedup 1.0000x reference)
"""Trainium2 Bass kernel for nn_Net_Jentzen_1network (dense_mlp, 8 cores).

Strategy
--------
The model is a 50-step scan. Per step:
    alpha = [t,x] @ Wa + ba
    h1 = relu(BN([t,x] @ W1 + b1));  h2 = relu(BN(h1 @ W2 + b2));  grad = h2@W3+b3
    f = 0.5*K*|x-law|^2 + 0.5*|alpha|^2
    v -= f*h;  v += <grad, s*xi>;  x += alpha*h + s*xi
plus a v0-net (2 BN layers) on the initial x.  BatchNorm uses FULL-batch
(B=2048) statistics each step, which couples the whole batch.

Key observation: the x-recursion does NOT depend on the BN tower (only on
alpha), and v is a pure accumulation.  So:

  Launch A (8 cores, data-parallel over batch): the serial x-path,
      x_{i+1}^T = A^T-matmul(x_i^T) + (s_i*xi_i^T + h_i*ba_eff_i)
      with A = I + h*Wa[1:] folded into one matmul (feature-major layout).
      Emits the transposed x-path.

  Launch B (8 cores, sharded over TIME STEPS): each core owns whole steps
      with the FULL batch, so BN stats are exact and core-local (zero
      collectives).  7 uniform SPMD slots per core; the v0-net runs as one
      slot of the same program (W6 zero-padded to 256 cols and a one-hot
      xi^T row, so the slot's "dot" output IS v0).

  Host: input transposes, the per-row f accumulation F (recovered exactly
      from the path identity alpha*h = x_{i+1}-x_i-s*xi), the b3 term of
      dot, and final assembly  v = v0 + b6 - F + sum_i dot_i + b3-term.

Everything on-device is fp32 (TensorE streams fp32 at the same col/cycle
rate as bf16, so fp32 costs nothing on the matmul-bound critical path).
"""

import os
import numpy as np

import concourse.bass as bass
import concourse.mybir as mybir
import concourse.tile as tile
from concourse import bacc
from concourse.bass_utils import run_bass_kernel_spmd

F32 = mybir.dt.float32
AF = mybir.ActivationFunctionType
ALU = mybir.AluOpType

DIM = 256
H = 356
HP = 384  # H padded to 3*128
B = 2048
T = 50
NCORES = 8
BSH = B // NCORES  # 256 batch rows per core in launch A
SLOTS = 7          # ceil(51 tasks / 8 cores)
KAPPA = 1.0
SIGMA = 0.3
BN_EPS = 1e-5

TRACE = bool(os.environ.get("KJ_TRACE"))
LAST_RESULTS = {}

_BUILT = {}


# ----------------------------------------------------------------------------
# Launch A: serial x-path recursion, data-parallel over batch (feature-major)
# ----------------------------------------------------------------------------
def build_launch_a():
    nc = bacc.Bacc("TRN2", target_bir_lowering=False, debug=False,
                   enable_asserts=False, num_devices=NCORES)
    A_d = nc.dram_tensor("A", [2, 128, DIM], F32, kind="ExternalInput").ap()
    x0_d = nc.dram_tensor("x0T", [2, 128, BSH], F32, kind="ExternalInput").ap()
    xi_d = nc.dram_tensor("xiTs", [T, 2, 128, BSH], F32, kind="ExternalInput").ap()
    hba_d = nc.dram_tensor("hba", [2, 128, T], F32, kind="ExternalInput").ap()
    path_d = nc.dram_tensor("xTpath", [T, 2, 128, BSH], F32, kind="ExternalOutput").ap()

    with tile.TileContext(nc) as tc:
        with (
            tc.tile_pool(name="singles", bufs=1) as singles,
            tc.tile_pool(name="xpool", bufs=3) as xpool,
            tc.tile_pool(name="xipool", bufs=4) as xipool,
            tc.tile_pool(name="upool", bufs=3) as upool,
            tc.tile_pool(name="psA", bufs=4, space="PSUM") as psA,
        ):
            A_sb = singles.tile([128, 2, DIM], F32)
            nc.sync.dma_start(A_sb, A_d.rearrange("o p m -> p o m"))
            hba_sb = singles.tile([128, 2, T], F32)
            nc.sync.dma_start(hba_sb, hba_d.rearrange("o p t -> p o t"))

            xT = xpool.tile([128, 2, BSH], F32, tag="x")
            nc.sync.dma_start(xT, x0_d.rearrange("o p b -> p o b"))

            for i in range(T):
                xi_t = xipool.tile([128, 2, BSH], F32, tag="xi")
                nc.sync.dma_start(xi_t, xi_d[i].rearrange("o p b -> p o b"))
                u = upool.tile([128, 2, BSH], F32, tag="u")
                for o in range(2):
                    # u = s_i*xi^T + h_i*ba_eff_i   (xi pre-scaled by s_i on host)
                    nc.vector.tensor_scalar(
                        u[:, o], xi_t[:, o],
                        scalar1=hba_sb[:, o, i:i + 1], scalar2=None,
                        op0=ALU.add)
                xT_new = xpool.tile([128, 2, BSH], F32, tag="x")
                for mo in range(2):
                    ps = psA.tile([128, BSH], F32, tag="ps")
                    for ko in range(2):
                        nc.tensor.matmul(
                            ps, A_sb[:, ko, mo * 128:(mo + 1) * 128], xT[:, ko],
                            start=(ko == 0), stop=(ko == 1))
                    nc.vector.tensor_tensor(xT_new[:, mo], ps, u[:, mo], ALU.add)
                nc.sync.dma_start(path_d[i].rearrange("o p b -> p o b"), xT_new)
                xT = xT_new
    nc.compile()
    return nc


# ----------------------------------------------------------------------------
# Launch B: BN tower, sharded over time steps (full batch per core)
# ----------------------------------------------------------------------------
def build_launch_b():
    nc = bacc.Bacc("TRN2", target_bir_lowering=False, debug=False,
                   enable_asserts=False, num_devices=NCORES)
    KH = HP // 128  # 3

    xT_d = nc.dram_tensor("xTin", [SLOTS, 2, 128, B], F32, kind="ExternalInput").ap()
    xi_d = nc.dram_tensor("xiTs", [SLOTS, 2, 128, B], F32, kind="ExternalInput").ap()
    wf_d = nc.dram_tensor("Wf", [SLOTS, 2, 128, HP], F32, kind="ExternalInput").ap()
    wm_d = nc.dram_tensor("Wm", [SLOTS, KH, 128, HP], F32, kind="ExternalInput").ap()
    wl_d = nc.dram_tensor("Wl", [SLOTS, KH, 128, DIM], F32, kind="ExternalInput").ap()
    g1_d = nc.dram_tensor("g1", [SLOTS, KH, 128], F32, kind="ExternalInput").ap()
    b1_d = nc.dram_tensor("be1", [SLOTS, KH, 128], F32, kind="ExternalInput").ap()
    g2_d = nc.dram_tensor("g2", [SLOTS, KH, 128], F32, kind="ExternalInput").ap()
    b2_d = nc.dram_tensor("be2", [SLOTS, KH, 128], F32, kind="ExternalInput").ap()
    dots_d = nc.dram_tensor("dots", [SLOTS, B], F32, kind="ExternalOutput").ap()

    with tile.TileContext(nc) as tc:
        with (
            tc.tile_pool(name="singles", bufs=1) as singles,
            tc.tile_pool(name="wpool", bufs=2) as wpool,
            tc.tile_pool(name="actpool", bufs=2) as actpool,
            tc.tile_pool(name="hpool", bufs=1) as hpool,
            tc.tile_pool(name="ppool", bufs=1) as ppool,
            tc.tile_pool(name="spool", bufs=8) as spool,
            tc.tile_pool(name="dpool", bufs=2) as dpool,
            tc.tile_pool(name="psB", bufs=2, space="PSUM") as psB,
        ):
            eps_sb = singles.tile([128, 1], F32)
            nc.vector.memset(eps_sb, BN_EPS)
            ones_sb = singles.tile([128, 1], F32)
            nc.vector.memset(ones_sb, 1.0)

            def bn_layer(ps, g_sb, b_sb, m, h_out):
                """stats of psum z-tile, then h_out[:,m,:] = relu(rg*z + b')."""
                stats = spool.tile([128, 4, 6], F32, tag="stats")
                for n in range(4):
                    nc.vector.bn_stats(stats[:, n], ps[:, n * 512:(n + 1) * 512])
                mv = spool.tile([128, 2], F32, tag="mv")
                nc.vector.bn_aggr(mv, stats)
                sd = spool.tile([128, 1], F32, tag="sd")
                nc.scalar.activation(sd, mv[:, 1:2], AF.Sqrt, bias=eps_sb, scale=1.0)
                r = spool.tile([128, 1], F32, tag="r")
                nc.vector.reciprocal(r, sd)
                rg = spool.tile([128, 1], F32, tag="rg")
                nc.vector.tensor_tensor(rg, r, g_sb[:, m:m + 1], ALU.mult)
                mrg = spool.tile([128, 1], F32, tag="mrg")
                nc.vector.tensor_tensor(mrg, mv[:, 0:1], rg, ALU.mult)
                bias_ = spool.tile([128, 1], F32, tag="bias")
                nc.vector.tensor_tensor(bias_, b_sb[:, m:m + 1], mrg, ALU.subtract)
                for n in range(4):
                    nc.scalar.activation(
                        h_out[:, m, n * 512:(n + 1) * 512],
                        ps[:, n * 512:(n + 1) * 512],
                        AF.Relu, bias=bias_, scale=rg)

            for slot in range(SLOTS):
                wf = wpool.tile([128, 2, HP], F32, tag="wf")
                nc.sync.dma_start(wf, wf_d[slot].rearrange("o p m -> p o m"))
                wm = wpool.tile([128, KH, HP], F32, tag="wm")
                nc.sync.dma_start(wm, wm_d[slot].rearrange("o p m -> p o m"))
                wl = wpool.tile([128, KH, DIM], F32, tag="wl")
                nc.sync.dma_start(wl, wl_d[slot].rearrange("o p m -> p o m"))
                g1s = wpool.tile([128, KH], F32, tag="g1")
                nc.sync.dma_start(g1s, g1_d[slot].rearrange("o p -> p o"))
                b1s = wpool.tile([128, KH], F32, tag="b1")
                nc.sync.dma_start(b1s, b1_d[slot].rearrange("o p -> p o"))
                g2s = wpool.tile([128, KH], F32, tag="g2")
                nc.sync.dma_start(g2s, g2_d[slot].rearrange("o p -> p o"))
                b2s = wpool.tile([128, KH], F32, tag="b2")
                nc.sync.dma_start(b2s, b2_d[slot].rearrange("o p -> p o"))

                xt = actpool.tile([128, 2, B], F32, tag="xt")
                nc.sync.dma_start(xt, xT_d[slot].rearrange("o p b -> p o b"))
                xi = actpool.tile([128, 2, B], F32, tag="xi")
                nc.sync.dma_start(xi, xi_d[slot].rearrange("o p b -> p o b"))

                # layer 1: z1 = x @ Wf  -> h1 = relu(BN(z1))
                h1 = hpool.tile([128, KH, B], F32, tag="h1")
                for m in range(KH):
                    ps = psB.tile([128, B], F32, tag="z")
                    for n in range(4):
                        nsl = slice(n * 512, (n + 1) * 512)
                        for k in range(2):
                            nc.tensor.matmul(
                                ps[:, nsl], wf[:, k, m * 128:(m + 1) * 128],
                                xt[:, k, nsl], start=(k == 0), stop=(k == 1))
                    bn_layer(ps, g1s, b1s, m, h1)

                # layer 2: z2 = h1 @ Wm -> h2 = relu(BN(z2))
                h2 = hpool.tile([128, KH, B], F32, tag="h2")
                for m in range(KH):
                    ps = psB.tile([128, B], F32, tag="z")
                    for n in range(4):
                        nsl = slice(n * 512, (n + 1) * 512)
                        for k in range(KH):
                            nc.tensor.matmul(
                                ps[:, nsl], wm[:, k, m * 128:(m + 1) * 128],
                                h1[:, k, nsl], start=(k == 0), stop=(k == KH - 1))
                    bn_layer(ps, g2s, b2s, m, h2)

                # grad = h2 @ Wl (no bias; b3 handled on host), then
                # prod = grad * (s*xi^T), dot = ones-reduce over features.
                prod = ppool.tile([128, 2, B], F32, tag="prod")
                for m in range(2):
                    ps = psB.tile([128, B], F32, tag="z")
                    for n in range(4):
                        nsl = slice(n * 512, (n + 1) * 512)
                        for k in range(KH):
                            nc.tensor.matmul(
                                ps[:, nsl], wl[:, k, m * 128:(m + 1) * 128],
                                h2[:, k, nsl], start=(k == 0), stop=(k == KH - 1))
                    nc.vector.tensor_tensor(prod[:, m], ps, xi[:, m], ALU.mult)

                dps = psB.tile([128, B], F32, tag="z")
                for n in range(4):
                    nsl = slice(n * 512, (n + 1) * 512)
                    for m in range(2):
                        nc.tensor.matmul(
                            dps[0:1, nsl], ones_sb[:, 0:1], prod[:, m, nsl],
                            start=(m == 0), stop=(m == 1))
                dot_sb = dpool.tile([1, B], F32, tag="dot")
                nc.vector.tensor_copy(dot_sb, dps[0:1, :])
                nc.sync.dma_start(dots_d[slot:slot + 1, :], dot_sb)
    nc.compile()
    return nc


def _get_programs():
    if "A" not in _BUILT:
        _BUILT["A"] = build_launch_a()
    if "B" not in _BUILT:
        _BUILT["B"] = build_launch_b()
    return _BUILT["A"], _BUILT["B"]


# ----------------------------------------------------------------------------
# Host orchestration
# ----------------------------------------------------------------------------
def _pad_w(w, rows, cols):
    out = np.zeros((rows, cols), np.float32)
    out[:w.shape[0], :w.shape[1]] = w
    return out


def _chunk_vec(v, total):
    """[<=total] -> [total//128, 128] zero-padded."""
    out = np.zeros((total,), np.float32)
    out[:v.shape[0]] = v
    return out.reshape(total // 128, 128)


def kernel(x, xi, law, timegrid, Wa, ba,
           W1, b1, g1, be1, W2, b2, g2, be2, W3, b3,
           W4, b4, g4, be4, W5, b5, g5, be5, W6, b6):
    x = np.asarray(x, np.float32)
    xi = np.asarray(xi, np.float32)
    law = np.asarray(law, np.float32)
    timegrid = np.asarray(timegrid, np.float32)

    t = timegrid[:-1]                      # [T]
    h = timegrid[1:] - timegrid[:-1]       # [T]
    s = (SIGMA * np.sqrt(h)).astype(np.float32)
    hbar = np.float32(h.mean())

    Wa1 = np.asarray(Wa, np.float32)[1:, :]            # [256,256]
    A = (np.eye(DIM, dtype=np.float32) + hbar * Wa1)   # [256,256]
    # per-step effective alpha bias times h: h_i*(ba + t_i*Wa[0,:])
    ba_eff = ba[None, :] + t[:, None] * Wa[0][None, :]          # [T,256]
    hba = (h[:, None] * ba_eff).astype(np.float32)              # [T,256]

    # transposed, s-scaled noise: xiTs[i] = s_i * xi_i^T   [T,256,B]
    xiTs = np.ascontiguousarray(np.transpose(xi, (0, 2, 1))) * s[:, None, None]
    xiTs = xiTs.astype(np.float32)
    x0T = np.ascontiguousarray(x.T)                             # [256,B]

    ncA, ncB = _get_programs()

    # ---------------- Launch A ----------------
    A_c = np.ascontiguousarray(A.reshape(2, 128, DIM))
    hba_c = np.ascontiguousarray(hba.T.reshape(2, 128, T))      # [2,128,T]
    in_maps_a = []
    for c in range(NCORES):
        bsl = slice(c * BSH, (c + 1) * BSH)
        in_maps_a.append(dict(
            A=A_c,
            x0T=np.ascontiguousarray(x0T[:, bsl].reshape(2, 128, BSH)),
            xiTs=np.ascontiguousarray(xiTs[:, :, bsl].reshape(T, 2, 128, BSH)),
            hba=hba_c,
        ))
    resA = run_bass_kernel_spmd(ncA, in_maps_a, core_ids=list(range(NCORES)),
                                trace=TRACE)
    LAST_RESULTS["A"] = resA

    # xTfull[i] = x_{i+1}^T as [T,2,128,B]
    xTfull = np.concatenate([r["xTpath"] for r in resA.results], axis=3)

    # path: [T+1, B, 256]
    path = np.empty((T + 1, B, DIM), np.float32)
    path[0] = x
    path[1:] = np.transpose(xTfull, (0, 3, 1, 2)).reshape(T, B, DIM)
    x_T = path[T]

    # ---------------- Launch B ----------------
    KH = HP // 128
    Wf_step = np.asarray(W1, np.float32)[1:, :]  # bias/t cancel in BN
    Wf_step = _pad_w(Wf_step, DIM, HP).reshape(2, 128, HP)
    Wm_step = _pad_w(np.asarray(W2, np.float32), HP, HP).reshape(KH, 128, HP)
    Wl_step = _pad_w(np.asarray(W3, np.float32), HP, DIM).reshape(KH, 128, DIM)
    Wf_v0 = _pad_w(np.asarray(W4, np.float32), DIM, HP).reshape(2, 128, HP)
    Wm_v0 = _pad_w(np.asarray(W5, np.float32), HP, HP).reshape(KH, 128, HP)
    Wl_v0 = _pad_w(np.asarray(W6, np.float32), HP, DIM).reshape(KH, 128, DIM)

    g1c, b1c = _chunk_vec(g1, HP), _chunk_vec(be1, HP)
    g2c, b2c = _chunk_vec(g2, HP), _chunk_vec(be2, HP)
    g4c, b4c = _chunk_vec(g4, HP), _chunk_vec(be4, HP)
    g5c, b5c = _chunk_vec(g5, HP), _chunk_vec(be5, HP)

    x0T_c = x0T.reshape(2, 128, B)
    xi_onehot = np.zeros((2, 128, B), np.float32)
    xi_onehot[0, 0, :] = 1.0
    xi_zero = np.zeros((2, 128, B), np.float32)

    # task list: 50 steps + v0, padded with dummies to 56
    tasks = [("step", i) for i in range(T)] + [("v0",)]
    tasks += [("dummy",)] * (NCORES * SLOTS - len(tasks))
    in_maps_b = []
    core_tasks = []
    for c in range(NCORES):
        tl = tasks[c * SLOTS:(c + 1) * SLOTS]
        core_tasks.append(tl)
        m = dict(xTin=np.empty((SLOTS, 2, 128, B), np.float32),
                 xiTs=np.empty((SLOTS, 2, 128, B), np.float32),
                 Wf=np.empty((SLOTS, 2, 128, HP), np.float32),
                 Wm=np.empty((SLOTS, KH, 128, HP), np.float32),
                 Wl=np.empty((SLOTS, KH, 128, DIM), np.float32),
                 g1=np.empty((SLOTS, KH, 128), np.float32),
                 be1=np.empty((SLOTS, KH, 128), np.float32),
                 g2=np.empty((SLOTS, KH, 128), np.float32),
                 be2=np.empty((SLOTS, KH, 128), np.float32))
        for j, tk in enumerate(tl):
            if tk[0] == "step":
                i = tk[1]
                m["xTin"][j] = x0T_c if i == 0 else xTfull[i - 1]
                m["xiTs"][j] = xiTs[i].reshape(2, 128, B)
                m["Wf"][j], m["Wm"][j], m["Wl"][j] = Wf_step, Wm_step, Wl_step
                m["g1"][j], m["be1"][j] = g1c, b1c
                m["g2"][j], m["be2"][j] = g2c, b2c
            elif tk[0] == "v0":
                m["xTin"][j] = x0T_c
                m["xiTs"][j] = xi_onehot
                m["Wf"][j], m["Wm"][j], m["Wl"][j] = Wf_v0, Wm_v0, Wl_v0
                m["g1"][j], m["be1"][j] = g4c, b4c
                m["g2"][j], m["be2"][j] = g5c, b5c
            else:
                m["xTin"][j] = x0T_c
                m["xiTs"][j] = xi_zero
                m["Wf"][j], m["Wm"][j], m["Wl"][j] = Wf_step, Wm_step, Wl_step
                m["g1"][j], m["be1"][j] = g1c, b1c
                m["g2"][j], m["be2"][j] = g2c, b2c
        in_maps_b.append(m)
    resB = run_bass_kernel_spmd(ncB, in_maps_b, core_ids=list(range(NCORES)),
                                trace=TRACE)
    LAST_RESULTS["B"] = resB

    dot_sum = np.zeros((B,), np.float64)
    v0 = None
    for c in range(NCORES):
        dots = resB.results[c]["dots"]
        for j, tk in enumerate(core_tasks[c]):
            if tk[0] == "step":
                dot_sum += dots[j]
            elif tk[0] == "v0":
                v0 = dots[j].astype(np.float64)
    assert v0 is not None

    # ---------------- host F and b3-term ----------------
    sxi = s[:, None, None] * xi                                  # [T,B,256]
    alphaH = path[1:] - path[:-1] - sxi                          # = h_i*alpha_i
    f_alpha = 0.5 * np.einsum("ibd,ibd->b", alphaH, alphaH / h[:, None, None])
    diff = path[:-1] - law[:, None, :]
    f_law = 0.5 * KAPPA * np.einsum("ibd,ibd,i->b", diff, diff, h)
    F = f_alpha + f_law
    dot_b3 = sxi.sum(axis=0) @ np.asarray(b3, np.float64)

    v = (v0 + np.float64(b6[0]) - F + dot_sum + dot_b3).astype(np.float32)
    return v[:, None], x_T, path


# revision 13
# speedup vs baseline: 1.5584x; 1.5584x over previous
"""Trainium2 Bass kernel for nn_Net_Jentzen_1network (dense_mlp, 8 cores).

Strategy
--------
The model is a 50-step scan. Per step:
    alpha = [t,x] @ Wa + ba
    h1 = relu(BN([t,x] @ W1 + b1));  h2 = relu(BN(h1 @ W2 + b2));  grad = h2@W3+b3
    f = 0.5*K*|x-law|^2 + 0.5*|alpha|^2
    v -= f*h;  v += <grad, s*xi>;  x += alpha*h + s*xi
plus a v0-net (2 BN layers) on the initial x.  BatchNorm uses FULL-batch
(B=2048) statistics each step, which couples the whole batch.

Key observation: the x-recursion does NOT depend on the BN tower (only on
alpha), and v is a pure accumulation.  So:

  Launch A (8 cores, data-parallel over batch): the serial x-path,
      x_{i+1}^T = A^T-matmul(x_i^T) + (s_i*xi_i^T + h_i*ba_eff_i)
      with A = I + h*Wa[1:] folded into one matmul (feature-major layout).
      Emits the transposed x-path.

  Launch B (8 cores, sharded over TIME STEPS): each core owns whole steps
      with the FULL batch, so BN stats are exact and core-local (zero
      collectives).  7 uniform SPMD slots per core; the v0-net runs as one
      slot of the same program (W6 zero-padded to 256 cols and a one-hot
      xi^T row, so the slot's "dot" output IS v0).

  Host: input transposes, the per-row f accumulation F (recovered exactly
      from the path identity alpha*h = x_{i+1}-x_i-s*xi), the b3 term of
      dot, and final assembly  v = v0 + b6 - F + sum_i dot_i + b3-term.

Everything on-device is fp32 (TensorE streams fp32 at the same col/cycle
rate as bf16, so fp32 costs nothing on the matmul-bound critical path).
"""

import os
import numpy as np

import concourse.bass as bass
import concourse.mybir as mybir
import concourse.tile as tile
from concourse import bacc
from concourse.bass_utils import run_bass_kernel_spmd

F32 = mybir.dt.float32
AF = mybir.ActivationFunctionType
ALU = mybir.AluOpType

DIM = 256
H = 356
HP = 384  # H padded to 3*128
B = 2048
T = 50
NCORES = 8
BSH = B // NCORES  # 256 batch rows per core in launch A
SLOTS = 7          # ceil(51 tasks / 8 cores)
KAPPA = 1.0
SIGMA = 0.3
BN_EPS = 1e-5

TRACE = bool(os.environ.get("KJ_TRACE"))
LAST_RESULTS = {}

_BUILT = {}


# ----------------------------------------------------------------------------
# Launch A: serial x-path recursion, data-parallel over batch (feature-major)
# ----------------------------------------------------------------------------
def build_launch_a():
    nc = bacc.Bacc("TRN2", target_bir_lowering=False, debug=False,
                   enable_asserts=False, num_devices=NCORES)
    # All big tensors partition-major [.., 128, 2, BSH] so each partition's
    # DMA row is one contiguous 2 KB chunk.
    A_d = nc.dram_tensor("A", [128, 2, DIM], F32, kind="ExternalInput").ap()
    x0_d = nc.dram_tensor("x0T", [128, 2, BSH], F32, kind="ExternalInput").ap()
    xi_d = nc.dram_tensor("xiTs", [T, 128, 2, BSH], F32, kind="ExternalInput").ap()
    hba_d = nc.dram_tensor("hba", [128, 2, T], F32, kind="ExternalInput").ap()
    path_d = nc.dram_tensor("xTpath", [T, 128, 2, BSH], F32, kind="ExternalOutput").ap()

    with tile.TileContext(nc) as tc:
        with (
            tc.tile_pool(name="singles", bufs=1) as singles,
            tc.tile_pool(name="xpool", bufs=3) as xpool,
            tc.tile_pool(name="xipool", bufs=4) as xipool,
            tc.tile_pool(name="upool", bufs=3) as upool,
            tc.tile_pool(name="psA", bufs=4, space="PSUM") as psA,
        ):
            A_sb = singles.tile([128, 2, DIM], F32)
            nc.sync.dma_start(A_sb, A_d)
            hba_sb = singles.tile([128, 2, T], F32)
            nc.sync.dma_start(hba_sb, hba_d)

            xT = xpool.tile([128, 2, BSH], F32, tag="x")
            nc.sync.dma_start(xT, x0_d)

            for i in range(T):
                xi_t = xipool.tile([128, 2, BSH], F32, tag="xi")
                nc.sync.dma_start(xi_t, xi_d[i])
                u = upool.tile([128, 2, BSH], F32, tag="u")
                for o in range(2):
                    # u = s_i*xi^T + h_i*ba_eff_i   (xi pre-scaled by s_i on host)
                    nc.vector.tensor_scalar(
                        u[:, o], xi_t[:, o],
                        scalar1=hba_sb[:, o, i:i + 1], scalar2=None,
                        op0=ALU.add)
                xT_new = xpool.tile([128, 2, BSH], F32, tag="x")
                for mo in range(2):
                    ps = psA.tile([128, BSH], F32, tag="ps")
                    for ko in range(2):
                        nc.tensor.matmul(
                            ps, A_sb[:, ko, mo * 128:(mo + 1) * 128], xT[:, ko],
                            start=(ko == 0), stop=(ko == 1))
                    nc.vector.tensor_tensor(xT_new[:, mo], ps, u[:, mo], ALU.add)
                nc.sync.dma_start(path_d[i], xT_new)
                xT = xT_new
    nc.compile()
    return nc


# ----------------------------------------------------------------------------
# Launch B: BN tower, sharded over time steps (full batch per core)
# ----------------------------------------------------------------------------
def build_launch_b():
    nc = bacc.Bacc("TRN2", target_bir_lowering=False, debug=False,
                   enable_asserts=False, num_devices=NCORES)
    KH = HP // 128  # 3
    F32R = mybir.dt.float32r

    xT_d = nc.dram_tensor("xTin", [SLOTS, 2, 128, B], F32R, kind="ExternalInput").ap()
    xi_d = nc.dram_tensor("xiTs", [SLOTS, 2, 128, B], F32, kind="ExternalInput").ap()
    wf_d = nc.dram_tensor("Wf", [SLOTS, 2, 128, HP], F32R, kind="ExternalInput").ap()
    wm_d = nc.dram_tensor("Wm", [SLOTS, KH, 128, HP], F32R, kind="ExternalInput").ap()
    wl_d = nc.dram_tensor("Wl", [SLOTS, KH, 128, DIM], F32R, kind="ExternalInput").ap()
    ones_d = nc.dram_tensor("ones", [128, 1], F32R, kind="ExternalInput").ap()
    g1_d = nc.dram_tensor("g1", [SLOTS, KH, 128], F32, kind="ExternalInput").ap()
    b1_d = nc.dram_tensor("be1", [SLOTS, KH, 128], F32, kind="ExternalInput").ap()
    g2_d = nc.dram_tensor("g2", [SLOTS, KH, 128], F32, kind="ExternalInput").ap()
    b2_d = nc.dram_tensor("be2", [SLOTS, KH, 128], F32, kind="ExternalInput").ap()
    dots_d = nc.dram_tensor("dots", [SLOTS, B], F32, kind="ExternalOutput").ap()

    with tile.TileContext(nc) as tc:
        with (
            tc.tile_pool(name="singles", bufs=1) as singles,
            tc.tile_pool(name="wpool", bufs=2) as wpool,
            tc.tile_pool(name="actpool", bufs=2) as actpool,
            tc.tile_pool(name="hpool", bufs=1) as hpool,
            tc.tile_pool(name="ppool", bufs=1) as ppool,
            tc.tile_pool(name="spool", bufs=8) as spool,
            tc.tile_pool(name="dpool", bufs=2) as dpool,
            tc.tile_pool(name="psB", bufs=2, space="PSUM") as psB,
        ):
            eps_sb = singles.tile([128, 1], F32)
            nc.vector.memset(eps_sb, BN_EPS)
            ones_sb = singles.tile([128, 1], F32R)
            nc.sync.dma_start(ones_sb, ones_d)

            def bn_layer(ps, g_sb, b_sb, m, h_out):
                """stats of psum z-tile, then h_out[:,m,:] = relu(rg*z + b')."""
                stats = spool.tile([128, 4, 6], F32, tag="stats")
                for n in range(4):
                    nc.vector.bn_stats(stats[:, n], ps[:, n * 512:(n + 1) * 512])
                mv = spool.tile([128, 2], F32, tag="mv")
                nc.vector.bn_aggr(mv, stats)
                sd = spool.tile([128, 1], F32, tag="sd")
                nc.scalar.activation(sd, mv[:, 1:2], AF.Sqrt, bias=eps_sb, scale=1.0)
                r = spool.tile([128, 1], F32, tag="r")
                nc.vector.reciprocal(r, sd)
                rg = spool.tile([128, 1], F32, tag="rg")
                nc.vector.tensor_tensor(rg, r, g_sb[:, m:m + 1], ALU.mult)
                mrg = spool.tile([128, 1], F32, tag="mrg")
                nc.vector.tensor_tensor(mrg, mv[:, 0:1], rg, ALU.mult)
                bias_ = spool.tile([128, 1], F32, tag="bias")
                nc.vector.tensor_tensor(bias_, b_sb[:, m:m + 1], mrg, ALU.subtract)
                nc.scalar.activation(h_out[:, m, :], ps[:, :],
                                     AF.Relu, bias=bias_, scale=rg)

            for slot in range(SLOTS):
                wf = wpool.tile([128, 2, HP], F32R, tag="wf")
                nc.sync.dma_start(wf, wf_d[slot].rearrange("o p m -> p o m"))
                wm = wpool.tile([128, KH, HP], F32R, tag="wm")
                nc.sync.dma_start(wm, wm_d[slot].rearrange("o p m -> p o m"))
                wl = wpool.tile([128, KH, DIM], F32R, tag="wl")
                nc.sync.dma_start(wl, wl_d[slot].rearrange("o p m -> p o m"))
                g1s = wpool.tile([128, KH], F32, tag="g1")
                nc.sync.dma_start(g1s, g1_d[slot].rearrange("o p -> p o"))
                b1s = wpool.tile([128, KH], F32, tag="b1")
                nc.sync.dma_start(b1s, b1_d[slot].rearrange("o p -> p o"))
                g2s = wpool.tile([128, KH], F32, tag="g2")
                nc.sync.dma_start(g2s, g2_d[slot].rearrange("o p -> p o"))
                b2s = wpool.tile([128, KH], F32, tag="b2")
                nc.sync.dma_start(b2s, b2_d[slot].rearrange("o p -> p o"))

                xt = actpool.tile([128, 2, B], F32R, tag="xt")
                nc.sync.dma_start(xt, xT_d[slot].rearrange("o p b -> p o b"))
                xi = actpool.tile([128, 2, B], F32, tag="xi")
                nc.sync.dma_start(xi, xi_d[slot].rearrange("o p b -> p o b"))

                # layer 1: z1 = x @ Wf  -> h1 = relu(BN(z1))
                h1 = hpool.tile([128, KH, B], F32R, tag="h1")
                for m in range(KH):
                    ps = psB.tile([128, B], F32, tag="z")
                    for k in range(2):
                        for n in range(4):
                            nsl = slice(n * 512, (n + 1) * 512)
                            nc.tensor.matmul(
                                ps[:, nsl], wf[:, k, m * 128:(m + 1) * 128],
                                xt[:, k, nsl], start=(k == 0), stop=(k == 1))
                    bn_layer(ps, g1s, b1s, m, h1)

                # layer 2: z2 = h1 @ Wm -> h2 = relu(BN(z2))
                h2 = hpool.tile([128, KH, B], F32R, tag="h2")
                for m in range(KH):
                    ps = psB.tile([128, B], F32, tag="z")
                    for k in range(KH):
                        for n in range(4):
                            nsl = slice(n * 512, (n + 1) * 512)
                            nc.tensor.matmul(
                                ps[:, nsl], wm[:, k, m * 128:(m + 1) * 128],
                                h1[:, k, nsl], start=(k == 0), stop=(k == KH - 1))
                    bn_layer(ps, g2s, b2s, m, h2)

                # grad = h2 @ Wl (no bias; b3 handled on host), then
                # prod = grad * (s*xi^T), dot = ones-reduce over features.
                prod = ppool.tile([128, 2, B], F32R, tag="prod")
                for m in range(2):
                    ps = psB.tile([128, B], F32, tag="z")
                    for k in range(KH):
                        for n in range(4):
                            nsl = slice(n * 512, (n + 1) * 512)
                            nc.tensor.matmul(
                                ps[:, nsl], wl[:, k, m * 128:(m + 1) * 128],
                                h2[:, k, nsl], start=(k == 0), stop=(k == KH - 1))
                    nc.vector.tensor_tensor(prod[:, m], ps, xi[:, m], ALU.mult)

                dps = psB.tile([128, B], F32, tag="z")
                for m in range(2):
                    for n in range(4):
                        nsl = slice(n * 512, (n + 1) * 512)
                        nc.tensor.matmul(
                            dps[0:1, nsl], ones_sb[:, 0:1], prod[:, m, nsl],
                            start=(m == 0), stop=(m == 1))
                dot_sb = dpool.tile([1, B], F32, tag="dot")
                nc.scalar.copy(dot_sb, dps[0:1, :])
                nc.sync.dma_start(dots_d[slot:slot + 1, :], dot_sb)
    nc.compile()
    return nc


def _get_programs():
    if "A" not in _BUILT:
        _BUILT["A"] = build_launch_a()
    if "B" not in _BUILT:
        _BUILT["B"] = build_launch_b()
    return _BUILT["A"], _BUILT["B"]


# ----------------------------------------------------------------------------
# Host orchestration
# ----------------------------------------------------------------------------
def _pad_w(w, rows, cols):
    out = np.zeros((rows, cols), np.float32)
    out[:w.shape[0], :w.shape[1]] = w
    return out


def _chunk_vec(v, total):
    """[<=total] -> [total//128, 128] zero-padded."""
    out = np.zeros((total,), np.float32)
    out[:v.shape[0]] = v
    return out.reshape(total // 128, 128)


def kernel(x, xi, law, timegrid, Wa, ba,
           W1, b1, g1, be1, W2, b2, g2, be2, W3, b3,
           W4, b4, g4, be4, W5, b5, g5, be5, W6, b6):
    x = np.asarray(x, np.float32)
    xi = np.asarray(xi, np.float32)
    law = np.asarray(law, np.float32)
    timegrid = np.asarray(timegrid, np.float32)

    t = timegrid[:-1]                      # [T]
    h = timegrid[1:] - timegrid[:-1]       # [T]
    s = (SIGMA * np.sqrt(h)).astype(np.float32)
    hbar = np.float32(h.mean())

    Wa1 = np.asarray(Wa, np.float32)[1:, :]            # [256,256]
    A = (np.eye(DIM, dtype=np.float32) + hbar * Wa1)   # [256,256]
    # per-step effective alpha bias times h: h_i*(ba + t_i*Wa[0,:])
    ba_eff = ba[None, :] + t[:, None] * Wa[0][None, :]          # [T,256]
    hba = (h[:, None] * ba_eff).astype(np.float32)              # [T,256]

    # transposed, s-scaled noise: xiTs[i] = s_i * xi_i^T   [T,256,B]
    xiTs = np.ascontiguousarray(np.transpose(xi, (0, 2, 1))) * s[:, None, None]
    xiTs = xiTs.astype(np.float32)
    x0T = np.ascontiguousarray(x.T)                             # [256,B]

    ncA, ncB = _get_programs()

    # ---------------- Launch A ----------------
    # p-major layouts: arr[p, o, ...] = full[o*128+p, ...]
    A_c = np.ascontiguousarray(A.reshape(2, 128, DIM).transpose(1, 0, 2))
    hba_c = np.ascontiguousarray(hba.T.reshape(2, 128, T).transpose(1, 0, 2))
    in_maps_a = []
    for c in range(NCORES):
        bsl = slice(c * BSH, (c + 1) * BSH)
        in_maps_a.append(dict(
            A=A_c,
            x0T=np.ascontiguousarray(
                x0T[:, bsl].reshape(2, 128, BSH).transpose(1, 0, 2)),
            xiTs=np.ascontiguousarray(
                xiTs[:, :, bsl].reshape(T, 2, 128, BSH).transpose(0, 2, 1, 3)),
            hba=hba_c,
        ))
    resA = run_bass_kernel_spmd(ncA, in_maps_a, core_ids=list(range(NCORES)),
                                trace=TRACE)
    LAST_RESULTS["A"] = resA

    # per-core output [T,128,2,BSH] (p-major) -> [T,2,128,B] (o-major, full batch)
    xTfull = np.concatenate(
        [r["xTpath"] for r in resA.results], axis=3).transpose(0, 2, 1, 3)
    xTfull = np.ascontiguousarray(xTfull)

    # path: [T+1, B, 256]
    path = np.empty((T + 1, B, DIM), np.float32)
    path[0] = x
    path[1:] = np.transpose(xTfull, (0, 3, 1, 2)).reshape(T, B, DIM)
    x_T = path[T]

    # ---------------- Launch B ----------------
    KH = HP // 128
    Wf_step = np.asarray(W1, np.float32)[1:, :]  # bias/t cancel in BN
    Wf_step = _pad_w(Wf_step, DIM, HP).reshape(2, 128, HP)
    Wm_step = _pad_w(np.asarray(W2, np.float32), HP, HP).reshape(KH, 128, HP)
    Wl_step = _pad_w(np.asarray(W3, np.float32), HP, DIM).reshape(KH, 128, DIM)
    Wf_v0 = _pad_w(np.asarray(W4, np.float32), DIM, HP).reshape(2, 128, HP)
    Wm_v0 = _pad_w(np.asarray(W5, np.float32), HP, HP).reshape(KH, 128, HP)
    Wl_v0 = _pad_w(np.asarray(W6, np.float32), HP, DIM).reshape(KH, 128, DIM)

    g1c, b1c = _chunk_vec(g1, HP), _chunk_vec(be1, HP)
    g2c, b2c = _chunk_vec(g2, HP), _chunk_vec(be2, HP)
    g4c, b4c = _chunk_vec(g4, HP), _chunk_vec(be4, HP)
    g5c, b5c = _chunk_vec(g5, HP), _chunk_vec(be5, HP)

    x0T_c = x0T.reshape(2, 128, B)
    xi_onehot = np.zeros((2, 128, B), np.float32)
    xi_onehot[0, 0, :] = 1.0
    xi_zero = np.zeros((2, 128, B), np.float32)

    # task list: 50 steps + v0, padded with dummies to 56
    tasks = [("step", i) for i in range(T)] + [("v0",)]
    tasks += [("dummy",)] * (NCORES * SLOTS - len(tasks))
    in_maps_b = []
    core_tasks = []
    for c in range(NCORES):
        tl = tasks[c * SLOTS:(c + 1) * SLOTS]
        core_tasks.append(tl)
        m = dict(xTin=np.empty((SLOTS, 2, 128, B), np.float32),
                 xiTs=np.empty((SLOTS, 2, 128, B), np.float32),
                 Wf=np.empty((SLOTS, 2, 128, HP), np.float32),
                 Wm=np.empty((SLOTS, KH, 128, HP), np.float32),
                 Wl=np.empty((SLOTS, KH, 128, DIM), np.float32),
                 g1=np.empty((SLOTS, KH, 128), np.float32),
                 be1=np.empty((SLOTS, KH, 128), np.float32),
                 g2=np.empty((SLOTS, KH, 128), np.float32),
                 be2=np.empty((SLOTS, KH, 128), np.float32),
                 ones=np.ones((128, 1), np.float32))
        for j, tk in enumerate(tl):
            if tk[0] == "step":
                i = tk[1]
                m["xTin"][j] = x0T_c if i == 0 else xTfull[i - 1]
                m["xiTs"][j] = xiTs[i].reshape(2, 128, B)
                m["Wf"][j], m["Wm"][j], m["Wl"][j] = Wf_step, Wm_step, Wl_step
                m["g1"][j], m["be1"][j] = g1c, b1c
                m["g2"][j], m["be2"][j] = g2c, b2c
            elif tk[0] == "v0":
                m["xTin"][j] = x0T_c
                m["xiTs"][j] = xi_onehot
                m["Wf"][j], m["Wm"][j], m["Wl"][j] = Wf_v0, Wm_v0, Wl_v0
                m["g1"][j], m["be1"][j] = g4c, b4c
                m["g2"][j], m["be2"][j] = g5c, b5c
            else:
                m["xTin"][j] = x0T_c
                m["xiTs"][j] = xi_zero
                m["Wf"][j], m["Wm"][j], m["Wl"][j] = Wf_step, Wm_step, Wl_step
                m["g1"][j], m["be1"][j] = g1c, b1c
                m["g2"][j], m["be2"][j] = g2c, b2c
        in_maps_b.append(m)
    resB = run_bass_kernel_spmd(ncB, in_maps_b, core_ids=list(range(NCORES)),
                                trace=TRACE)
    LAST_RESULTS["B"] = resB

    dot_sum = np.zeros((B,), np.float64)
    v0 = None
    for c in range(NCORES):
        dots = resB.results[c]["dots"]
        for j, tk in enumerate(core_tasks[c]):
            if tk[0] == "step":
                dot_sum += dots[j]
            elif tk[0] == "v0":
                v0 = dots[j].astype(np.float64)
    assert v0 is not None

    # ---------------- host F and b3-term ----------------
    sxi = s[:, None, None] * xi                                  # [T,B,256]
    alphaH = path[1:] - path[:-1] - sxi                          # = h_i*alpha_i
    f_alpha = 0.5 * np.einsum("ibd,ibd->b", alphaH, alphaH / h[:, None, None])
    diff = path[:-1] - law[:, None, :]
    f_law = 0.5 * KAPPA * np.einsum("ibd,ibd,i->b", diff, diff, h)
    F = f_alpha + f_law
    dot_b3 = sxi.sum(axis=0) @ np.asarray(b3, np.float64)

    v = (v0 + np.float64(b6[0]) - F + dot_sum + dot_b3).astype(np.float32)
    return v[:, None], x_T, path


# revision 25
# speedup vs baseline: 1.9219x; 1.2332x over previous
"""Trainium2 Bass kernel for nn_Net_Jentzen_1network (dense_mlp, 8 cores).

Strategy
--------
The model is a 50-step scan. Per step:
    alpha = [t,x] @ Wa + ba
    h1 = relu(BN([t,x] @ W1 + b1));  h2 = relu(BN(h1 @ W2 + b2));  grad = h2@W3+b3
    f = 0.5*K*|x-law|^2 + 0.5*|alpha|^2
    v -= f*h;  v += <grad, s*xi>;  x += alpha*h + s*xi
plus a v0-net (2 BN layers) on the initial x.  BatchNorm uses FULL-batch
(B=2048) statistics each step, which couples the whole batch.

Key observation: the x-recursion does NOT depend on the BN tower (only on
alpha), and v is a pure accumulation.  So:

  Launch A (8 cores, data-parallel over batch): the serial x-path,
      x_{i+1}^T = A^T-matmul(x_i^T) + (s_i*xi_i^T + h_i*ba_eff_i)
      with A = I + h*Wa[1:] folded into one matmul (feature-major layout).
      Emits the transposed x-path.

  Launch B (8 cores, sharded over TIME STEPS): each core owns whole steps
      with the FULL batch, so BN stats are exact and core-local (zero
      collectives).  7 uniform SPMD slots per core; the v0-net runs as one
      slot of the same program (W6 zero-padded to 256 cols and a one-hot
      xi^T row, so the slot's "dot" output IS v0).

  Host: input transposes, the per-row f accumulation F (recovered exactly
      from the path identity alpha*h = x_{i+1}-x_i-s*xi), the b3 term of
      dot, and final assembly  v = v0 + b6 - F + sum_i dot_i + b3-term.

Everything on-device is fp32 (TensorE streams fp32 at the same col/cycle
rate as bf16, so fp32 costs nothing on the matmul-bound critical path).
"""

import os
import numpy as np

import concourse.bass as bass
import concourse.mybir as mybir
import concourse.tile as tile
from concourse import bacc
from concourse.bass_utils import run_bass_kernel_spmd

F32 = mybir.dt.float32
AF = mybir.ActivationFunctionType
ALU = mybir.AluOpType

DIM = 256
H = 356
HP = 384  # H padded to 3*128
B = 2048
T = 50
NCORES = 8
BSH = B // NCORES  # 256 batch rows per core in launch A
SLOTS = 7          # ceil(51 tasks / 8 cores)
KAPPA = 1.0
SIGMA = 0.3
BN_EPS = 1e-5

TRACE = bool(os.environ.get("KJ_TRACE"))
LAST_RESULTS = {}

_BUILT = {}


# ----------------------------------------------------------------------------
# Launch A: serial x-path recursion, data-parallel over batch (feature-major)
# ----------------------------------------------------------------------------
GA = 5          # steps per DMA batch in launch A
NGA = T // GA   # 10


def build_launch_a():
    """x-recursion, data-parallel over batch, feature-major.

    x_{i+1}^T = A^T-matmul(x_i^T) + u_i  where u_i = s_i*xi_i^T + h_i*ba_eff_i
    is fully host-precomputed (in the xiTs input).  The two 128-column batch
    halves are independent chains and are interleaved so PE(half0) overlaps
    DVE(half1).  xi loads and path stores are batched GA steps per DMA.
    """
    nc = bacc.Bacc("TRN2", target_bir_lowering=False, debug=False,
                   enable_asserts=False, num_devices=NCORES)
    A_d = nc.dram_tensor("A", [128, 2, DIM], F32, kind="ExternalInput").ap()
    x0_d = nc.dram_tensor("x0T", [128, 2, BSH], F32, kind="ExternalInput").ap()
    xi_d = nc.dram_tensor("xiTs", [NGA, 128, GA, 2, BSH], F32,
                          kind="ExternalInput").ap()
    path_d = nc.dram_tensor("xTpath", [NGA, 128, GA, 2, BSH], F32,
                            kind="ExternalOutput").ap()

    with tile.TileContext(nc) as tc:
        with (
            tc.tile_pool(name="singles", bufs=1) as singles,
            tc.tile_pool(name="xpool", bufs=2) as xpool,
            tc.tile_pool(name="stpool", bufs=2) as stpool,
            tc.tile_pool(name="xipool", bufs=3) as xipool,
            tc.tile_pool(name="psA", bufs=8, space="PSUM") as psA,
        ):
            A_sb = singles.tile([128, 2, DIM], F32)
            nc.sync.dma_start(A_sb, A_d)

            x0 = xpool.tile([128, 2, BSH], F32, tag="x0")
            nc.sync.dma_start(x0, x0_d)

            xT = x0  # current state AP (readable as [:, o, b])
            for g in range(NGA):
                xi_t = xipool.tile([128, GA, 2, BSH], F32, tag="xi")
                nc.sync.dma_start(xi_t, xi_d[g])
                stage = stpool.tile([128, GA, 2, BSH], F32, tag="st")
                for j in range(GA):
                    xT_new = stage[:, j]
                    for hh in range(2):
                        hsl = slice(hh * 128, (hh + 1) * 128)
                        ps = psA.tile([128, 2, 128], F32, tag="ps")
                        for mo in range(2):
                            for ko in range(2):
                                nc.tensor.matmul(
                                    ps[:, mo],
                                    A_sb[:, ko, mo * 128:(mo + 1) * 128],
                                    xT[:, ko, hsl],
                                    start=(ko == 0), stop=(ko == 1))
                        nc.vector.tensor_tensor(
                            xT_new[:, :, hsl], ps, xi_t[:, j, :, hsl], ALU.add)
                    xT = xT_new
                nc.sync.dma_start(path_d[g], stage)
    nc.compile()
    return nc


# ----------------------------------------------------------------------------
# Launch B: BN tower, sharded over time steps (full batch per core)
# ----------------------------------------------------------------------------
def build_launch_b():
    nc = bacc.Bacc("TRN2", target_bir_lowering=False, debug=False,
                   enable_asserts=False, num_devices=NCORES)
    KH = HP // 128  # 3
    F32R = mybir.dt.float32r

    xT_d = nc.dram_tensor("xTin", [SLOTS, 2, 128, B], F32R, kind="ExternalInput").ap()
    xi_d = nc.dram_tensor("xiTs", [SLOTS, 2, 128, B], F32, kind="ExternalInput").ap()
    wf_d = nc.dram_tensor("Wf", [SLOTS, 2, 128, HP], F32R, kind="ExternalInput").ap()
    wm_d = nc.dram_tensor("Wm", [SLOTS, KH, 128, HP], F32R, kind="ExternalInput").ap()
    wl_d = nc.dram_tensor("Wl", [SLOTS, KH, 128, DIM], F32R, kind="ExternalInput").ap()
    ones_d = nc.dram_tensor("ones", [128, 1], F32R, kind="ExternalInput").ap()
    g1_d = nc.dram_tensor("g1", [SLOTS, KH, 128], F32, kind="ExternalInput").ap()
    b1_d = nc.dram_tensor("be1", [SLOTS, KH, 128], F32, kind="ExternalInput").ap()
    g2_d = nc.dram_tensor("g2", [SLOTS, KH, 128], F32, kind="ExternalInput").ap()
    b2_d = nc.dram_tensor("be2", [SLOTS, KH, 128], F32, kind="ExternalInput").ap()
    dots_d = nc.dram_tensor("dots", [SLOTS, B], F32, kind="ExternalOutput").ap()

    with tile.TileContext(nc) as tc:
        with (
            tc.tile_pool(name="singles", bufs=1) as singles,
            tc.tile_pool(name="wpool", bufs=2) as wpool,
            tc.tile_pool(name="actpool", bufs=2) as actpool,
            tc.tile_pool(name="hpool", bufs=2) as hpool,
            tc.tile_pool(name="ppool", bufs=1) as ppool,
            tc.tile_pool(name="spool", bufs=8) as spool,
            tc.tile_pool(name="dpool", bufs=1) as dpool,
            tc.tile_pool(name="psB", bufs=4, space="PSUM") as psB,
        ):
            eps_sb = singles.tile([128, 1], F32)
            nc.vector.memset(eps_sb, BN_EPS)
            ones_sb = singles.tile([128, 1], F32R)
            nc.sync.dma_start(ones_sb, ones_d)

            HB = B // 2  # 1024: psum tiles are batch-halves (2 banks each)

            def bn_layer(pss, g_sb, b_sb, m, h_out):
                """stats over both half-psums, then h = relu(rg*z + b')."""
                stats = spool.tile([128, 4, 6], F32, tag="stats")
                for hh in range(2):
                    for n in range(2):
                        nc.vector.bn_stats(
                            stats[:, hh * 2 + n],
                            pss[hh][:, n * 512:(n + 1) * 512])
                mv = spool.tile([128, 2], F32, tag="mv")
                nc.vector.bn_aggr(mv, stats)
                sd = spool.tile([128, 1], F32, tag="sd")
                nc.scalar.activation(sd, mv[:, 1:2], AF.Sqrt, bias=eps_sb, scale=1.0)
                r = spool.tile([128, 1], F32, tag="r")
                nc.vector.reciprocal(r, sd)
                rg = spool.tile([128, 1], F32, tag="rg")
                nc.vector.tensor_tensor(rg, r, g_sb[:, m:m + 1], ALU.mult)
                mrg = spool.tile([128, 1], F32, tag="mrg")
                nc.vector.tensor_tensor(mrg, mv[:, 0:1], rg, ALU.mult)
                bias_ = spool.tile([128, 1], F32, tag="bias")
                nc.vector.tensor_tensor(bias_, b_sb[:, m:m + 1], mrg, ALU.subtract)
                for hh in range(2):
                    nc.scalar.activation(h_out[:, m, hh * HB:(hh + 1) * HB],
                                         pss[hh][:, :],
                                         AF.Relu, bias=bias_, scale=rg)

            def mm_layer(w_sb, nk, act_in, m):
                """returns 2 half-psum tiles for out-chunk m of a layer."""
                pss = []
                for hh in range(2):
                    ps = psB.tile([128, HB], F32, tag="z")
                    for k in range(nk):
                        for n in range(2):
                            nsl = slice(hh * HB + n * 512, hh * HB + (n + 1) * 512)
                            psl = slice(n * 512, (n + 1) * 512)
                            nc.tensor.matmul(
                                ps[:, psl], w_sb[:, k, m * 128:(m + 1) * 128],
                                act_in[:, k, nsl],
                                start=(k == 0), stop=(k == nk - 1))
                    pss.append(ps)
                return pss

            for slot in range(SLOTS):
                wf = wpool.tile([128, 2, HP], F32R, tag="wf")
                nc.sync.dma_start(wf, wf_d[slot].rearrange("o p m -> p o m"))
                wm = wpool.tile([128, KH, HP], F32R, tag="wm")
                nc.sync.dma_start(wm, wm_d[slot].rearrange("o p m -> p o m"))
                wl = wpool.tile([128, KH, DIM], F32R, tag="wl")
                nc.sync.dma_start(wl, wl_d[slot].rearrange("o p m -> p o m"))
                g1s = wpool.tile([128, KH], F32, tag="g1")
                nc.sync.dma_start(g1s, g1_d[slot].rearrange("o p -> p o"))
                b1s = wpool.tile([128, KH], F32, tag="b1")
                nc.sync.dma_start(b1s, b1_d[slot].rearrange("o p -> p o"))
                g2s = wpool.tile([128, KH], F32, tag="g2")
                nc.sync.dma_start(g2s, g2_d[slot].rearrange("o p -> p o"))
                b2s = wpool.tile([128, KH], F32, tag="b2")
                nc.sync.dma_start(b2s, b2_d[slot].rearrange("o p -> p o"))

                xt = actpool.tile([128, 2, B], F32R, tag="xt")
                nc.sync.dma_start(xt, xT_d[slot].rearrange("o p b -> p o b"))
                xi = actpool.tile([128, 2, B], F32, tag="xi")
                nc.sync.dma_start(xi, xi_d[slot].rearrange("o p b -> p o b"))

                # layer 1: z1 = x @ Wf  -> h1 = relu(BN(z1))
                h1 = hpool.tile([128, KH, B], F32R, tag="h1")
                for m in range(KH):
                    bn_layer(mm_layer(wf, 2, xt, m), g1s, b1s, m, h1)

                # layer 2: z2 = h1 @ Wm -> h2 = relu(BN(z2))
                h2 = hpool.tile([128, KH, B], F32R, tag="h2")
                for m in range(KH):
                    bn_layer(mm_layer(wm, KH, h1, m), g2s, b2s, m, h2)

                # grad = h2 @ Wl (no bias; b3 handled on host), then
                # prod = grad * (s*xi^T), dot = ones-reduce over features.
                prod = ppool.tile([128, 2, B], F32R, tag="prod")
                for m in range(2):
                    pss = mm_layer(wl, KH, h2, m)
                    for hh in range(2):
                        nc.vector.tensor_tensor(
                            prod[:, m, hh * HB:(hh + 1) * HB], pss[hh],
                            xi[:, m, hh * HB:(hh + 1) * HB], ALU.mult)

                dot_sb = dpool.tile([1, B], F32, tag="dot")
                for hh in range(2):
                    dps = psB.tile([1, HB], F32, tag="z")
                    for m in range(2):
                        for n in range(2):
                            nsl = slice(hh * HB + n * 512, hh * HB + (n + 1) * 512)
                            psl = slice(n * 512, (n + 1) * 512)
                            nc.tensor.matmul(
                                dps[:, psl], ones_sb[:, 0:1], prod[:, m, nsl],
                                start=(m == 0), stop=(m == 1))
                    nc.scalar.copy(dot_sb[:, hh * HB:(hh + 1) * HB], dps)
                nc.sync.dma_start(dots_d[slot:slot + 1, :], dot_sb)
    nc.compile()
    return nc


def _get_program(which):
    if which not in _BUILT:
        _BUILT[which] = build_launch_a() if which == "A" else build_launch_b()
    return _BUILT[which]


# ----------------------------------------------------------------------------
# Host orchestration
# ----------------------------------------------------------------------------
def _pad_w(w, rows, cols):
    out = np.zeros((rows, cols), np.float32)
    out[:w.shape[0], :w.shape[1]] = w
    return out


def _chunk_vec(v, total):
    """[<=total] -> [total//128, 128] zero-padded."""
    out = np.zeros((total,), np.float32)
    out[:v.shape[0]] = v
    return out.reshape(total // 128, 128)


def kernel(x, xi, law, timegrid, Wa, ba,
           W1, b1, g1, be1, W2, b2, g2, be2, W3, b3,
           W4, b4, g4, be4, W5, b5, g5, be5, W6, b6):
    x = np.asarray(x, np.float32)
    xi = np.asarray(xi, np.float32)
    law = np.asarray(law, np.float32)
    timegrid = np.asarray(timegrid, np.float32)

    t = timegrid[:-1]                      # [T]
    h = timegrid[1:] - timegrid[:-1]       # [T]
    s = (SIGMA * np.sqrt(h)).astype(np.float32)
    hbar = np.float32(h.mean())

    Wa1 = np.asarray(Wa, np.float32)[1:, :]            # [256,256]
    A = (np.eye(DIM, dtype=np.float32) + hbar * Wa1)   # [256,256]
    # per-step effective alpha bias times h: h_i*(ba + t_i*Wa[0,:])
    ba_eff = ba[None, :] + t[:, None] * Wa[0][None, :]          # [T,256]
    hba = (h[:, None] * ba_eff).astype(np.float32)              # [T,256]

    # transposed, s-scaled noise: xiTs[i] = s_i * xi_i^T   [T,256,B]
    xiTs = (np.ascontiguousarray(np.transpose(xi, (0, 2, 1)))
            * s[:, None, None]).astype(np.float32)
    # launch-A step increment u_i = s_i*xi_i^T + h_i*ba_eff_i
    uA = (xiTs + hba[:, :, None]).astype(np.float32)
    x0T = np.ascontiguousarray(x.T)                             # [256,B]

    ncA = _get_program("A")

    # ---------------- Launch A ----------------
    # p-major layouts: arr[p, o, ...] = full[o*128+p, ...]; xi/path further
    # grouped GA steps per DMA: [NGA, 128, GA, 2, BSH]
    A_c = np.ascontiguousarray(A.reshape(2, 128, DIM).transpose(1, 0, 2))
    in_maps_a = []
    for c in range(NCORES):
        bsl = slice(c * BSH, (c + 1) * BSH)
        in_maps_a.append(dict(
            A=A_c,
            x0T=np.ascontiguousarray(
                x0T[:, bsl].reshape(2, 128, BSH).transpose(1, 0, 2)),
            xiTs=np.ascontiguousarray(
                uA[:, :, bsl].reshape(NGA, GA, 2, 128, BSH)
                .transpose(0, 3, 1, 2, 4)),
        ))
    resA = run_bass_kernel_spmd(ncA, in_maps_a, core_ids=list(range(NCORES)),
                                trace=TRACE)
    LAST_RESULTS["A"] = resA

    # per-core output [NGA,128,GA,2,BSH] (p-major) -> [T,2,128,B] (full batch)
    xTfull = np.concatenate(
        [r["xTpath"] for r in resA.results], axis=4)          # [NGA,128,GA,2,B]
    xTfull = np.ascontiguousarray(
        xTfull.transpose(0, 2, 3, 1, 4)).reshape(T, 2, 128, B)

    # path: [T+1, B, 256]
    path = np.empty((T + 1, B, DIM), np.float32)
    path[0] = x
    path[1:] = np.transpose(xTfull, (0, 3, 1, 2)).reshape(T, B, DIM)
    x_T = path[T]

    # ---------------- Launch B ----------------
    KH = HP // 128
    Wf_step = np.asarray(W1, np.float32)[1:, :]  # bias/t cancel in BN
    Wf_step = _pad_w(Wf_step, DIM, HP).reshape(2, 128, HP)
    Wm_step = _pad_w(np.asarray(W2, np.float32), HP, HP).reshape(KH, 128, HP)
    Wl_step = _pad_w(np.asarray(W3, np.float32), HP, DIM).reshape(KH, 128, DIM)
    Wf_v0 = _pad_w(np.asarray(W4, np.float32), DIM, HP).reshape(2, 128, HP)
    Wm_v0 = _pad_w(np.asarray(W5, np.float32), HP, HP).reshape(KH, 128, HP)
    Wl_v0 = _pad_w(np.asarray(W6, np.float32), HP, DIM).reshape(KH, 128, DIM)

    g1c, b1c = _chunk_vec(g1, HP), _chunk_vec(be1, HP)
    g2c, b2c = _chunk_vec(g2, HP), _chunk_vec(be2, HP)
    g4c, b4c = _chunk_vec(g4, HP), _chunk_vec(be4, HP)
    g5c, b5c = _chunk_vec(g5, HP), _chunk_vec(be5, HP)

    x0T_c = x0T.reshape(2, 128, B)
    xi_onehot = np.zeros((2, 128, B), np.float32)
    xi_onehot[0, 0, :] = 1.0
    xi_zero = np.zeros((2, 128, B), np.float32)

    # task list: 50 steps + v0, padded with dummies to 56
    tasks = [("step", i) for i in range(T)] + [("v0",)]
    tasks += [("dummy",)] * (NCORES * SLOTS - len(tasks))
    in_maps_b = []
    core_tasks = []
    for c in range(NCORES):
        tl = tasks[c * SLOTS:(c + 1) * SLOTS]
        core_tasks.append(tl)
        m = dict(xTin=np.empty((SLOTS, 2, 128, B), np.float32),
                 xiTs=np.empty((SLOTS, 2, 128, B), np.float32),
                 Wf=np.empty((SLOTS, 2, 128, HP), np.float32),
                 Wm=np.empty((SLOTS, KH, 128, HP), np.float32),
                 Wl=np.empty((SLOTS, KH, 128, DIM), np.float32),
                 g1=np.empty((SLOTS, KH, 128), np.float32),
                 be1=np.empty((SLOTS, KH, 128), np.float32),
                 g2=np.empty((SLOTS, KH, 128), np.float32),
                 be2=np.empty((SLOTS, KH, 128), np.float32),
                 ones=np.ones((128, 1), np.float32))
        for j, tk in enumerate(tl):
            if tk[0] == "step":
                i = tk[1]
                m["xTin"][j] = x0T_c if i == 0 else xTfull[i - 1]
                m["xiTs"][j] = xiTs[i].reshape(2, 128, B)
                m["Wf"][j], m["Wm"][j], m["Wl"][j] = Wf_step, Wm_step, Wl_step
                m["g1"][j], m["be1"][j] = g1c, b1c
                m["g2"][j], m["be2"][j] = g2c, b2c
            elif tk[0] == "v0":
                m["xTin"][j] = x0T_c
                m["xiTs"][j] = xi_onehot
                m["Wf"][j], m["Wm"][j], m["Wl"][j] = Wf_v0, Wm_v0, Wl_v0
                m["g1"][j], m["be1"][j] = g4c, b4c
                m["g2"][j], m["be2"][j] = g5c, b5c
            else:
                m["xTin"][j] = x0T_c
                m["xiTs"][j] = xi_zero
                m["Wf"][j], m["Wm"][j], m["Wl"][j] = Wf_step, Wm_step, Wl_step
                m["g1"][j], m["be1"][j] = g1c, b1c
                m["g2"][j], m["be2"][j] = g2c, b2c
        in_maps_b.append(m)
    resB = run_bass_kernel_spmd(_get_program("B"), in_maps_b,
                                core_ids=list(range(NCORES)),
                                trace=TRACE)
    LAST_RESULTS["B"] = resB

    dot_sum = np.zeros((B,), np.float64)
    v0 = None
    for c in range(NCORES):
        dots = resB.results[c]["dots"]
        for j, tk in enumerate(core_tasks[c]):
            if tk[0] == "step":
                dot_sum += dots[j]
            elif tk[0] == "v0":
                v0 = dots[j].astype(np.float64)
    assert v0 is not None

    # ---------------- host F and b3-term ----------------
    sxi = s[:, None, None] * xi                                  # [T,B,256]
    alphaH = path[1:] - path[:-1] - sxi                          # = h_i*alpha_i
    f_alpha = 0.5 * np.einsum("ibd,ibd->b", alphaH, alphaH / h[:, None, None])
    diff = path[:-1] - law[:, None, :]
    f_law = 0.5 * KAPPA * np.einsum("ibd,ibd,i->b", diff, diff, h)
    F = f_alpha + f_law
    dot_b3 = sxi.sum(axis=0) @ np.asarray(b3, np.float64)

    v = (v0 + np.float64(b6[0]) - F + dot_sum + dot_b3).astype(np.float32)
    return v[:, None], x_T, path
